# revision 3
# baseline (speedup 1.0000x reference)
"""Trainium2 Bass kernel for nn_AttentionDecoder (Bahdanau attention over T=32768).

Math: scores = enc2d @ w_enc + (dec_h @ w_dec + b); nw = softmax(scores);
      output = nw @ enc2d.
Softmax is shift-invariant, so the scalar (dec_h @ w_dec + b) term cancels and
dec_h / w_dec / attn_b never need to touch the device. Scores are N(0, ~0.64),
so exp() needs no max-subtraction for stability.

Sharding: enc is split along T across 8 cores (4096 rows each). Each core
 - loads its chunk in natural layout [t on partitions], t = 32*p + r
 - scores via fused DVE multiply+reduce against a replicated w_enc
 - expw = exp(scores) on ACT
 - partial weighted sum A = expw.T @ enc via PE (PSUM-accumulated)
 - partial denominator E = sum(expw)
 - AllGather{A, E} (4KB/rank), then normalizes on-device:
   output = sum_r A_r / sum_r E_r, nw_chunk = expw / sum_r E_r.
"""
import os
import sys

if '/opt/trn_rl_repo' not in sys.path:
    sys.path.insert(0, '/opt/trn_rl_repo')

import numpy as np

N_CORES = 8
T, H, O = 32768, 1024, 1024
TC = T // N_CORES          # 4096 rows per core
NB = TC // 128             # 32 blocks of 128 rows; t_local = 32*p + b
MODE = os.environ.get("BASS_ATTN_MODE", "device")  # device | host

_cache = {}


def _build(mode):
    import concourse.bass as bass
    import concourse.mybir as mybir
    from concourse import bacc, tile

    f32 = mybir.dt.float32
    Alu = mybir.AluOpType
    Act = mybir.ActivationFunctionType

    nc = bacc.Bacc("TRN2", debug=False, target_bir_lowering=False,
                   num_devices=N_CORES)

    enc_t = nc.dram_tensor("enc_chunk", [TC, H], f32, kind="ExternalInput")
    w_t = nc.dram_tensor("w_rep", [128, H], f32, kind="ExternalInput")
    nw_out = nc.dram_tensor("nw_chunk", [128, NB], f32, kind="ExternalOutput")
    if mode == "device":
        out_vec = nc.dram_tensor("out_vec", [1, O], f32, kind="ExternalOutput")
    else:
        pa_out = nc.dram_tensor("pa_out", [1, 1056], f32, kind="ExternalOutput")

    enc3 = enc_t.ap().rearrange("(p r) h -> p r h", r=NB)  # [128, 32, 1024]

    with tile.TileContext(nc) as tc:
        with (
            tc.tile_pool(name="nat", bufs=8) as natp,
            tc.tile_pool(name="scr", bufs=2) as scrp,
            tc.tile_pool(name="small", bufs=1) as smp,
            tc.tile_pool(name="psum", bufs=1, space="PSUM") as psp,
            tc.tile_pool(name="dram", bufs=1, space="DRAM") as drp,
        ):
            wt = smp.tile([128, H], f32)
            nc.sync.dma_start(wt[:], w_t.ap())

            scores = smp.tile([128, NB], f32)
            expw = smp.tile([128, NB], f32)
            psum_a = psp.tile([1, 1024], f32)
            psum_e = psp.tile([1, 1], f32)
            ones128 = smp.tile([128, 1], f32)
            nc.gpsimd.memset(ones128[:], 1.0)

            # PE warmup: ~4.5us of dummy matmuls under the first DMA so the
            # HAM clock-gate reaches 2.4 GHz before the real accumulation.
            psum_warm = psp.tile([1, 128], f32)
            for _ in range(42):
                nc.tensor.matmul(psum_warm[0:1, :], lhsT=ones128[:],
                                 rhs=wt[:, 0:128], start=True, stop=True)

            # 8 DMAs of 2 MiB; group g holds blocks [4g, 4g+4)
            for g in range(8):
                nat = natp.tile([128, 4, H], f32, tag="nat")
                nc.sync.dma_start(nat[:], enc3[:, 4 * g:4 * g + 4, :])
                for dr in range(4):
                    b = 4 * g + dr
                    dummy = scrp.tile([128, 1], f32, tag="dm")
                    # scores[:, b] = sum_h enc[t, h] * w[h]  (fused DVE op;
                    # elementwise product discarded via broadcast dummy)
                    nc.vector.affine_mul_reduce(
                        out=dummy[:].broadcast_to((128, H)),
                        accum_out=scores[:, b:b + 1],
                        in0=nat[:, dr, :], in1=wt[:], scale=1.0, bias=0.0)
                nc.scalar.activation(expw[:, 4 * g:4 * g + 4],
                                     scores[:, 4 * g:4 * g + 4], Act.Exp)
                for dr in range(4):
                    b = 4 * g + dr
                    for half in range(2):
                        nc.tensor.matmul(
                            psum_a[0:1, 512 * half:512 * half + 512],
                            lhsT=expw[:, b:b + 1],
                            rhs=nat[:, dr, 512 * half:512 * half + 512],
                            start=(b == 0), stop=(b == NB - 1))

            # E = sum over all expw
            ecol = smp.tile([128, 1], f32)
            nc.vector.reduce_sum(ecol[:], expw[:], axis=mybir.AxisListType.X)
            nc.tensor.matmul(psum_e[0:1, 0:1], lhsT=ones128[:], rhs=ecol[:],
                             start=True, stop=True)

            blob = smp.tile([1, 1056], f32)
            nc.gpsimd.memset(blob[:, 1025:1056], 0.0)
            nc.vector.tensor_copy(blob[0:1, 0:1024], psum_a[0:1, :])
            nc.vector.tensor_copy(blob[0:1, 1024:1025], psum_e[0:1, :])

            if mode == "host":
                nc.sync.dma_start(pa_out.ap(), blob[:])
                nc.sync.dma_start(nw_out.ap(), expw[:])
            else:
                cc_in = drp.tile([1, 1056], f32)
                cc_out = drp.tile([1, 1056 * N_CORES], f32, addr_space="Shared")
                nc.sync.dma_start(cc_in[:], blob[:])
                nc.gpsimd.collective_compute(
                    "AllGather", Alu.bypass,
                    replica_groups=[list(range(N_CORES))],
                    ins=[cc_in[:].opt()], outs=[cc_out[:].opt()])
                gath = smp.tile([8, 1056], f32)
                nc.sync.dma_start(
                    gath[:], cc_out[:].rearrange("a (c n) -> a c n", c=8)[0])

                ones8 = smp.tile([8, 128], f32)
                nc.gpsimd.memset(ones8[:], 1.0)
                psum_d = psp.tile([128, 1], f32)
                nc.tensor.matmul(psum_d[:], lhsT=ones8[:],
                                 rhs=gath[0:8, 1024:1025], start=True, stop=True)
                psum_o = psp.tile([1, 1024], f32)
                for half in range(2):
                    nc.tensor.matmul(
                        psum_o[0:1, 512 * half:512 * half + 512],
                        lhsT=ones8[0:8, 0:1],
                        rhs=gath[0:8, 512 * half:512 * half + 512],
                        start=True, stop=True)

                recip = smp.tile([128, 1], f32)
                nc.vector.reciprocal(recip[:], psum_d[:])
                out_sb = smp.tile([1, 1024], f32)
                nc.vector.tensor_scalar_mul(out_sb[:], psum_o[0:1, :],
                                            recip[0:1, 0:1])
                nw_sb = smp.tile([128, NB], f32)
                nc.vector.tensor_scalar_mul(nw_sb[:], expw[:], recip[:])
                nc.sync.dma_start(out_vec.ap(), out_sb[:])
                nc.sync.dma_start(nw_out.ap(), nw_sb[:])

    nc.compile()
    return nc


def _get_nc(mode):
    if mode not in _cache:
        _cache[mode] = _build(mode)
    return _cache[mode]


def _in_maps(enc, attn_w):
    enc2d = np.ascontiguousarray(np.asarray(enc, dtype=np.float32).reshape(T, H))
    w_enc = np.asarray(attn_w, dtype=np.float32).reshape(-1)[O:O + H]
    w_rep = np.ascontiguousarray(np.broadcast_to(w_enc, (128, H)))
    return [
        {"enc_chunk": enc2d[c * TC:(c + 1) * TC], "w_rep": w_rep}
        for c in range(N_CORES)
    ]


def run_spmd(enc, attn_w, mode=MODE, trace=False):
    """Build+run the SPMD kernel; returns BassKernelResults."""
    from concourse.bass_utils import run_bass_kernel_spmd
    nc = _get_nc(mode)
    return run_bass_kernel_spmd(nc, _in_maps(enc, attn_w),
                                core_ids=list(range(N_CORES)), trace=trace)


def _finalize(res, mode):
    if mode == "host":
        a = np.stack([res.results[c]["pa_out"][0] for c in range(N_CORES)])
        denom = a[:, 1024].sum(dtype=np.float64)
        output = (a[:, :1024].sum(axis=0, dtype=np.float64) / denom)
        output = output.astype(np.float32).reshape(1, O)
        nw = np.concatenate(
            [res.results[c]["nw_chunk"].reshape(-1) for c in range(N_CORES)])
        nw = (nw / np.float32(denom)).astype(np.float32).reshape(1, T)
    else:
        output = np.asarray(res.results[0]["out_vec"]).reshape(1, O)
        nw = np.concatenate(
            [res.results[c]["nw_chunk"].reshape(-1) for c in range(N_CORES)]
        ).reshape(1, T)
    return output, nw


def kernel(dec_h, enc, attn_w, attn_b):
    res = run_spmd(enc, attn_w, mode=MODE, trace=False)
    return _finalize(res, MODE)


# revision 8
# speedup vs baseline: 1.1964x; 1.1964x over previous
"""Trainium2 Bass kernel for nn_AttentionDecoder (Bahdanau attention over T=32768).

Math: scores = enc2d @ w_enc + (dec_h @ w_dec + b); nw = softmax(scores);
      output = nw @ enc2d.
Softmax is shift-invariant, so the scalar (dec_h @ w_dec + b) term cancels and
dec_h / w_dec / attn_b never need to touch the device. Scores are N(0, ~0.64),
so exp() needs no max-subtraction for stability.

Sharding: enc is split along T across 8 cores (4096 rows each). Each core
 - loads its chunk in natural layout [t on partitions], t = 32*p + r
 - scores via fused DVE multiply+reduce against a replicated w_enc
 - expw = exp(scores) on ACT
 - partial weighted sum A = expw.T @ enc via PE (PSUM-accumulated)
 - partial denominator E = sum(expw)
 - AllGather{A, E} (4KB/rank), then normalizes on-device:
   output = sum_r A_r / sum_r E_r, nw_chunk = expw / sum_r E_r.
"""
import os
import sys

if '/opt/trn_rl_repo' not in sys.path:
    sys.path.insert(0, '/opt/trn_rl_repo')

import numpy as np

N_CORES = 8
T, H, O = 32768, 1024, 1024
TC = T // N_CORES          # 4096 rows per core
NB = TC // 128             # 32 blocks of 128 rows; t_local = 32*p + b
MODE = os.environ.get("BASS_ATTN_MODE", "device")  # device | host

_cache = {}


def _build(mode):
    import concourse.bass as bass
    import concourse.mybir as mybir
    from concourse import bacc, tile

    f32 = mybir.dt.float32
    f16 = mybir.dt.float16
    Alu = mybir.AluOpType
    Act = mybir.ActivationFunctionType

    nc = bacc.Bacc("TRN2", debug=False, target_bir_lowering=False,
                   num_devices=N_CORES)

    enc_t = nc.dram_tensor("enc_chunk", [TC, H], f32, kind="ExternalInput")
    w_t = nc.dram_tensor("w_rep", [128, H], f32, kind="ExternalInput")
    nw_out = nc.dram_tensor("nw_chunk", [128, NB], f32, kind="ExternalOutput")
    if mode == "device":
        out_vec = nc.dram_tensor("out_vec", [1, O], f32, kind="ExternalOutput")
    else:
        pa_out = nc.dram_tensor("pa_out", [1, 1056], f32, kind="ExternalOutput")

    enc3 = enc_t.ap().rearrange("(p r) h -> p r h", r=NB)  # [128, 32, 1024]

    with tile.TileContext(nc) as tc:
        with (
            tc.tile_pool(name="nat", bufs=4) as natp,
            tc.tile_pool(name="nat16", bufs=4) as n16p,
            tc.tile_pool(name="scr", bufs=2) as scrp,
            tc.tile_pool(name="small", bufs=1) as smp,
            tc.tile_pool(name="psum", bufs=1, space="PSUM") as psp,
            tc.tile_pool(name="dram", bufs=1, space="DRAM") as drp,
        ):
            wt = smp.tile([128, H], f32)
            nc.sync.dma_start(wt[:], w_t.ap())

            scores = smp.tile([128, NB], f32)
            expw = smp.tile([128, NB], f32)
            expw16 = smp.tile([128, NB], f16)
            psum_a = psp.tile([1, 1024], f32)
            psum_e = psp.tile([1, 1], f32)
            ones128 = smp.tile([128, 1], f32)
            nc.gpsimd.memset(ones128[:], 1.0)

            # 8 DMAs of 2 MiB; group g holds blocks [4g, 4g+4)
            for g in range(8):
                nat = natp.tile([128, 4, H], f32, tag="nat")
                nc.sync.dma_start(nat[:], enc3[:, 4 * g:4 * g + 4, :])
                # fp16 shadow copy for the PE pass (f32 MMs stream twice;
                # fp16 once). Cast on the otherwise-idle ACT engine.
                nat16 = n16p.tile([128, 4, H], f16, tag="n16")
                nc.scalar.activation(nat16[:], nat[:], Act.Copy)
                for dr in range(4):
                    b = 4 * g + dr
                    dummy = scrp.tile([128, 1], f32, tag="dm")
                    # scores[:, b] = sum_h enc[t, h] * w[h]  (fused DVE op;
                    # elementwise product discarded via broadcast dummy)
                    nc.vector.affine_mul_reduce(
                        out=dummy[:].broadcast_to((128, H)),
                        accum_out=scores[:, b:b + 1],
                        in0=nat[:, dr, :], in1=wt[:], scale=1.0, bias=0.0)
                nc.scalar.activation(expw[:, 4 * g:4 * g + 4],
                                     scores[:, 4 * g:4 * g + 4], Act.Exp)
                nc.scalar.activation(expw16[:, 4 * g:4 * g + 4],
                                     expw[:, 4 * g:4 * g + 4], Act.Copy)
                for dr in range(4):
                    b = 4 * g + dr
                    for half in range(2):
                        nc.tensor.matmul(
                            psum_a[0:1, 512 * half:512 * half + 512],
                            lhsT=expw16[:, b:b + 1],
                            rhs=nat16[:, dr, 512 * half:512 * half + 512],
                            start=(b == 0), stop=(b == NB - 1))

            # E = sum over all expw
            ecol = smp.tile([128, 1], f32)
            nc.vector.reduce_sum(ecol[:], expw[:], axis=mybir.AxisListType.X)
            nc.tensor.matmul(psum_e[0:1, 0:1], lhsT=ones128[:], rhs=ecol[:],
                             start=True, stop=True)

            blob = smp.tile([1, 1056], f32)
            nc.gpsimd.memset(blob[:, 1025:1056], 0.0)
            nc.vector.tensor_copy(blob[0:1, 0:1024], psum_a[0:1, :])
            nc.vector.tensor_copy(blob[0:1, 1024:1025], psum_e[0:1, :])

            if mode == "host":
                nc.sync.dma_start(pa_out.ap(), blob[:])
                nc.sync.dma_start(nw_out.ap(), expw[:])
            else:
                cc_in = drp.tile([1, 1056], f32)
                cc_out = drp.tile([1, 1056 * N_CORES], f32, addr_space="Shared")
                nc.sync.dma_start(cc_in[:], blob[:])
                nc.gpsimd.collective_compute(
                    "AllGather", Alu.bypass,
                    replica_groups=[list(range(N_CORES))],
                    ins=[cc_in[:].opt()], outs=[cc_out[:].opt()])
                gath = smp.tile([8, 1056], f32)
                nc.sync.dma_start(
                    gath[:], cc_out[:].rearrange("a (c n) -> a c n", c=8)[0])

                ones8 = smp.tile([8, 128], f32)
                nc.gpsimd.memset(ones8[:], 1.0)
                psum_d = psp.tile([128, 1], f32)
                nc.tensor.matmul(psum_d[:], lhsT=ones8[:],
                                 rhs=gath[0:8, 1024:1025], start=True, stop=True)
                psum_o = psp.tile([1, 1024], f32)
                for half in range(2):
                    nc.tensor.matmul(
                        psum_o[0:1, 512 * half:512 * half + 512],
                        lhsT=ones8[0:8, 0:1],
                        rhs=gath[0:8, 512 * half:512 * half + 512],
                        start=True, stop=True)

                recip = smp.tile([128, 1], f32)
                nc.vector.reciprocal(recip[:], psum_d[:])
                out_sb = smp.tile([1, 1024], f32)
                nc.vector.tensor_scalar_mul(out_sb[:], psum_o[0:1, :],
                                            recip[0:1, 0:1])
                nw_sb = smp.tile([128, NB], f32)
                nc.vector.tensor_scalar_mul(nw_sb[:], expw[:], recip[:])
                nc.sync.dma_start(out_vec.ap(), out_sb[:])
                nc.sync.dma_start(nw_out.ap(), nw_sb[:])

    nc.compile()
    return nc


def _get_nc(mode):
    if mode not in _cache:
        _cache[mode] = _build(mode)
    return _cache[mode]


def _in_maps(enc, attn_w):
    enc2d = np.ascontiguousarray(np.asarray(enc, dtype=np.float32).reshape(T, H))
    w_enc = np.asarray(attn_w, dtype=np.float32).reshape(-1)[O:O + H]
    w_rep = np.ascontiguousarray(np.broadcast_to(w_enc, (128, H)))
    return [
        {"enc_chunk": enc2d[c * TC:(c + 1) * TC], "w_rep": w_rep}
        for c in range(N_CORES)
    ]


def run_spmd(enc, attn_w, mode=MODE, trace=False):
    """Build+run the SPMD kernel; returns BassKernelResults."""
    from concourse.bass_utils import run_bass_kernel_spmd
    nc = _get_nc(mode)
    return run_bass_kernel_spmd(nc, _in_maps(enc, attn_w),
                                core_ids=list(range(N_CORES)), trace=trace)


def _finalize(res, mode):
    if mode == "host":
        a = np.stack([res.results[c]["pa_out"][0] for c in range(N_CORES)])
        denom = a[:, 1024].sum(dtype=np.float64)
        output = (a[:, :1024].sum(axis=0, dtype=np.float64) / denom)
        output = output.astype(np.float32).reshape(1, O)
        nw = np.concatenate(
            [res.results[c]["nw_chunk"].reshape(-1) for c in range(N_CORES)])
        nw = (nw / np.float32(denom)).astype(np.float32).reshape(1, T)
    else:
        output = np.asarray(res.results[0]["out_vec"]).reshape(1, O)
        nw = np.concatenate(
            [res.results[c]["nw_chunk"].reshape(-1) for c in range(N_CORES)]
        ).reshape(1, T)
    return output, nw


def kernel(dec_h, enc, attn_w, attn_b):
    res = run_spmd(enc, attn_w, mode=MODE, trace=False)
    return _finalize(res, MODE)


# revision 11
# speedup vs baseline: 2.0058x; 1.6765x over previous
"""Trainium2 Bass kernel for nn_AttentionDecoder (Bahdanau attention over T=32768).

Math: scores = enc2d @ w_enc + (dec_h @ w_dec + b); nw = softmax(scores);
      output = nw @ enc2d.
Softmax is shift-invariant, so the scalar (dec_h @ w_dec + b) term cancels and
dec_h / w_dec / attn_b never need to touch the device. Scores are N(0, ~0.64),
so exp() needs no max-subtraction for stability.

Sharding: enc is split along T across 8 cores (4096 rows each). Each core
 - loads its chunk in natural layout [t on partitions], t = 32*p + r
 - scores via fused DVE multiply+reduce against a replicated w_enc
 - expw = exp(scores) on ACT
 - partial weighted sum A = expw.T @ enc via PE (PSUM-accumulated)
 - partial denominator E = sum(expw)
 - AllGather{A, E} (4KB/rank), then normalizes on-device:
   output = sum_r A_r / sum_r E_r, nw_chunk = expw / sum_r E_r.
"""
import os
import sys

if '/opt/trn_rl_repo' not in sys.path:
    sys.path.insert(0, '/opt/trn_rl_repo')

import numpy as np

N_CORES = 8
T, H, O = 32768, 1024, 1024
TC = T // N_CORES          # 4096 rows per core
NB = TC // 128             # 32 blocks of 128 rows; t_local = 32*p + b
MODE = os.environ.get("BASS_ATTN_MODE", "device")  # device | host

_cache = {}


def _build(mode):
    import concourse.bass as bass
    import concourse.mybir as mybir
    from concourse import bacc, tile

    f32 = mybir.dt.float32
    f16 = mybir.dt.float16
    Alu = mybir.AluOpType
    Act = mybir.ActivationFunctionType

    nc = bacc.Bacc("TRN2", debug=False, target_bir_lowering=False,
                   num_devices=N_CORES)

    enc_t = nc.dram_tensor("enc_chunk", [TC, H], f32, kind="ExternalInput")
    w_t = nc.dram_tensor("w_rep", [128, H], f32, kind="ExternalInput")
    nw_out = nc.dram_tensor("nw_chunk", [128, NB], f32, kind="ExternalOutput")
    if mode == "device":
        out_vec = nc.dram_tensor("out_vec", [1, O], f32, kind="ExternalOutput")
    else:
        pa_out = nc.dram_tensor("pa_out", [1, 1056], f32, kind="ExternalOutput")

    enc3 = enc_t.ap().rearrange("(p r) h -> p r h", r=NB)  # [128, 32, 1024]

    with tile.TileContext(nc) as tc:
        with (
            tc.tile_pool(name="nat", bufs=4) as natp,
            tc.tile_pool(name="nat16", bufs=4) as n16p,
            tc.tile_pool(name="scr", bufs=2) as scrp,
            tc.tile_pool(name="small", bufs=1) as smp,
            tc.tile_pool(name="psum", bufs=1, space="PSUM") as psp,
            tc.tile_pool(name="dram", bufs=1, space="DRAM") as drp,
        ):
            wt = smp.tile([128, H], f32)
            nc.sync.dma_start(wt[:], w_t.ap())

            scores = smp.tile([128, NB], f32)
            expw = smp.tile([128, NB], f32)
            expw16 = smp.tile([128, NB], f16)
            psum_a = psp.tile([1, 1024], f32)
            psum_e = psp.tile([1, 1], f32)
            ones128 = smp.tile([128, 1], f32)
            nc.gpsimd.memset(ones128[:], 1.0)

            # 2 MiB DMA groups; the last group is split into single-block
            # DMAs so the final score-reduce chain starts sooner.
            groups = [(4 * g, 4) for g in range(7)] + [(28 + i, 1) for i in range(4)]
            for b0, nb in groups:
                nat = natp.tile([128, 4, H], f32, tag="nat")
                nc.sync.dma_start(nat[:, 0:nb], enc3[:, b0:b0 + nb, :])
                # fp16 shadow copy for the PE pass (f32 MMs stream twice;
                # fp16 once). Cast on the otherwise-idle ACT engine.
                nat16 = n16p.tile([128, 4, H], f16, tag="n16")
                nc.scalar.activation(nat16[:, 0:nb], nat[:, 0:nb], Act.Copy)
                for dr in range(nb):
                    b = b0 + dr
                    dummy = scrp.tile([128, 1], f32, tag="dm")
                    # scores[:, b] = sum_h enc[t, h] * w[h]  (fused DVE op;
                    # elementwise product discarded via broadcast dummy)
                    nc.vector.affine_mul_reduce(
                        out=dummy[:].broadcast_to((128, H)),
                        accum_out=scores[:, b:b + 1],
                        in0=nat[:, dr, :], in1=wt[:], scale=1.0, bias=0.0)
                nc.scalar.activation(expw[:, b0:b0 + nb],
                                     scores[:, b0:b0 + nb], Act.Exp)
                nc.scalar.activation(expw16[:, b0:b0 + nb],
                                     expw[:, b0:b0 + nb], Act.Copy)
                for dr in range(nb):
                    b = b0 + dr
                    for half in range(2):
                        nc.tensor.matmul(
                            psum_a[0:1, 512 * half:512 * half + 512],
                            lhsT=expw16[:, b:b + 1],
                            rhs=nat16[:, dr, 512 * half:512 * half + 512],
                            start=(b == 0), stop=(b == NB - 1))

            # E = sum over all expw
            ecol = smp.tile([128, 1], f32)
            nc.vector.reduce_sum(ecol[:], expw[:], axis=mybir.AxisListType.X)
            nc.tensor.matmul(psum_e[0:1, 0:1], lhsT=ones128[:], rhs=ecol[:],
                             start=True, stop=True)

            blob = smp.tile([1, 1056], f32)
            nc.gpsimd.memset(blob[:, 1025:1056], 0.0)
            # PSUM -> SBUF staging on ACT (DVE is busy with the last reduces)
            nc.scalar.activation(blob[0:1, 0:1024], psum_a[0:1, :], Act.Copy)
            nc.vector.tensor_copy(blob[0:1, 1024:1025], psum_e[0:1, :])

            if mode == "host":
                nc.sync.dma_start(pa_out.ap(), blob[:])
                nc.sync.dma_start(nw_out.ap(), expw[:])
            else:
                cc_in = drp.tile([1, 1056], f32)
                cc_out = drp.tile([1, 1056 * N_CORES], f32, addr_space="Shared")
                nc.sync.dma_start(cc_in[:], blob[:])
                nc.gpsimd.collective_compute(
                    "AllGather", Alu.bypass,
                    replica_groups=[list(range(N_CORES))],
                    ins=[cc_in[:].opt()], outs=[cc_out[:].opt()])
                gath = smp.tile([8, 1056], f32)
                nc.sync.dma_start(
                    gath[:], cc_out[:].rearrange("a (c n) -> a c n", c=8)[0])

                ones8 = smp.tile([8, 128], f32)
                nc.gpsimd.memset(ones8[:], 1.0)
                psum_d = psp.tile([128, 1], f32)
                nc.tensor.matmul(psum_d[:], lhsT=ones8[:],
                                 rhs=gath[0:8, 1024:1025], start=True, stop=True)
                psum_o = psp.tile([1, 1024], f32)
                for half in range(2):
                    nc.tensor.matmul(
                        psum_o[0:1, 512 * half:512 * half + 512],
                        lhsT=ones8[0:8, 0:1],
                        rhs=gath[0:8, 512 * half:512 * half + 512],
                        start=True, stop=True)

                recip = smp.tile([128, 1], f32)
                nc.vector.reciprocal(recip[:], psum_d[:])
                out_sb = smp.tile([1, 1024], f32)
                nc.vector.tensor_scalar_mul(out_sb[:], psum_o[0:1, :],
                                            recip[0:1, 0:1])
                nw_sb = smp.tile([128, NB], f32)
                nc.vector.tensor_scalar_mul(nw_sb[:], expw[:], recip[:])
                nc.sync.dma_start(out_vec.ap(), out_sb[:])
                nc.sync.dma_start(nw_out.ap(), nw_sb[:])

    nc.compile()
    return nc


def _get_nc(mode):
    if mode not in _cache:
        _cache[mode] = _build(mode)
    return _cache[mode]


def _in_maps(enc, attn_w):
    enc2d = np.ascontiguousarray(np.asarray(enc, dtype=np.float32).reshape(T, H))
    w_enc = np.asarray(attn_w, dtype=np.float32).reshape(-1)[O:O + H]
    w_rep = np.ascontiguousarray(np.broadcast_to(w_enc, (128, H)))
    return [
        {"enc_chunk": enc2d[c * TC:(c + 1) * TC], "w_rep": w_rep}
        for c in range(N_CORES)
    ]


def run_spmd(enc, attn_w, mode=MODE, trace=False):
    """Build+run the SPMD kernel; returns BassKernelResults."""
    from concourse.bass_utils import run_bass_kernel_spmd
    nc = _get_nc(mode)
    return run_bass_kernel_spmd(nc, _in_maps(enc, attn_w),
                                core_ids=list(range(N_CORES)), trace=trace)


def _finalize(res, mode):
    if mode == "host":
        a = np.stack([res.results[c]["pa_out"][0] for c in range(N_CORES)])
        denom = a[:, 1024].sum(dtype=np.float64)
        output = (a[:, :1024].sum(axis=0, dtype=np.float64) / denom)
        output = output.astype(np.float32).reshape(1, O)
        nw = np.concatenate(
            [res.results[c]["nw_chunk"].reshape(-1) for c in range(N_CORES)])
        nw = (nw / np.float32(denom)).astype(np.float32).reshape(1, T)
    else:
        output = np.asarray(res.results[0]["out_vec"]).reshape(1, O)
        nw = np.concatenate(
            [res.results[c]["nw_chunk"].reshape(-1) for c in range(N_CORES)]
        ).reshape(1, T)
    return output, nw


def kernel(dec_h, enc, attn_w, attn_b):
    res = run_spmd(enc, attn_w, mode=MODE, trace=False)
    return _finalize(res, MODE)
